# revision 1
# baseline (speedup 1.0000x reference)
"""ODE-RNN decoder kernel for Trainium2 (8 NeuronCores, data-parallel).

Math per scan step (t = 0..98), per trajectory:
    y_ode = y + (tanh(y @ Wo1 + bo1) @ Wo2 + bo2) * dt_t
    z     = sigmoid(tanh([y_ode;x] @ Wz1 + bz1) @ Wz2 + bz2)
    r     = sigmoid(tanh([y_ode;x] @ Wr1 + br1) @ Wr2 + br2)
    h     = tanh(tanh([r*y_ode;x] @ Wh1 + bh1) @ Wh2 + bh2)
    y     = (1-z)*h + z*y_ode

Layout: feature-major on-chip ([feature, batch]); batch 8192 sharded 8 ways
data-parallel (1024/core, weights replicated), each core processing CH=3
independent chunks so their recurrence chains pipeline through the engines.
Matmuls run in float32r (TF32-like, ~1.3e-4 input rounding, 1 cycle/row at
N>=256; free dim must be a multiple of 8) except the z-gate layer 2, which
needs a base-64 PSUM destination (packed [z;r] sigmoid) that fp32r can't
write — it uses bf16. The [y;x] gate concat is avoided by splitting each
layer-1 weight into x-rows and y-rows and accumulating two matmuls into one
PSUM tile; layer-1 biases ride the tanh activations, bo2 rides a ones-row
appended to the ODE hidden activations.
"""

import sys

sys.path.insert(0, "/opt/trn_rl_repo")

from contextlib import ExitStack

import numpy as np

import concourse.bass as bass
import concourse.tile as tile
from concourse import bacc, mybir
from concourse.bass_utils import run_bass_kernel_spmd

N_TRAJ, T, DD, DL, NU = 8192, 100, 32, 64, 100
NSTEP = T - 1
NCORES = 8
B = N_TRAJ // NCORES  # 1024 per core
import os
CH = int(os.environ.get("KCH", "3"))   # chunks in flight per core
PS_L1 = int(os.environ.get("KPSL1", "4"))
PS_B = int(os.environ.get("KPSB", "4"))


def _chunk_bounds():
    """Split B into CH chunks, each a multiple of 8 (fp32r matmul free-dim
    restriction), summing to B."""
    blocks = B // 8
    per = [blocks // CH + (1 if i < blocks % CH else 0) for i in range(CH)]
    sizes = [p * 8 for p in per]
    offs = [0]
    for s_ in sizes:
        offs.append(offs[-1] + s_)
    return sizes, offs


CHS, COFF = _chunk_bounds()
NCH = max(CHS)

F32 = mybir.dt.float32
F32R = mybir.dt.float32r
TANH = mybir.ActivationFunctionType.Tanh
SIG = mybir.ActivationFunctionType.Sigmoid
ADD = mybir.AluOpType.add
MULT = mybir.AluOpType.mult


def _build():
    nc = bacc.Bacc("TRN2", target_bir_lowering=False, debug=False)

    def din(name, shape, dt=F32R):
        return nc.dram_tensor(name, list(shape), dt, kind="ExternalInput")

    xs = din("xs", [NSTEP, DD, B])         # host-transposed data[:,1:,:]
    prior = din("prior", [DL, B])
    wo1 = din("wo1", [DL, NU])
    wo2b = din("wo2b", [NU + 1, DL])       # [Wo2; bo2] (ones-row trick)
    wz1y = din("wz1y", [DL, NU]); wz1x = din("wz1x", [DD, NU])
    wr1y = din("wr1y", [DL, NU]); wr1x = din("wr1x", [DD, NU])
    wh1y = din("wh1y", [DL, NU]); wh1x = din("wh1x", [DD, NU])
    BF16 = mybir.dt.bfloat16
    wz2 = din("wz2", [NU, DL], BF16)   # z2 writes PSUM base 64: f32r can't
    wr2 = din("wr2", [NU, DL]); wh2 = din("wh2", [NU, DL])
    bo1 = din("bo1", [NU, 1], F32); bz1 = din("bz1", [NU, 1], F32)
    br1 = din("br1", [NU, 1], F32); bh1 = din("bh1", [NU, 1], F32)
    bzr2 = din("bzr2", [2 * DL, 1], F32)   # [bz2; br2]
    bh2 = din("bh2", [DL, 1], F32)
    dts = din("dts", [DL, NSTEP], F32)     # dt broadcast over partitions
    sgn = din("sgn", [2 * DL, 1], F32)     # [+1]*64 + [-1]*64 sigmoid scales
    ones = din("ones", [32, NCH])          # f32r ones rows for wo2b bias row
    yout = nc.dram_tensor("yout", [DL, B], F32R, kind="ExternalOutput")

    mmul = nc.tensor.matmul

    with tile.TileContext(nc) as tc, ExitStack() as ctx:
        singles = ctx.enter_context(tc.tile_pool(name="singles", bufs=1))
        xpool = ctx.enter_context(tc.tile_pool(name="xp", bufs=16))
        _l1 = ctx.enter_context(tc.tile_pool(name="ps_l1", bufs=PS_L1, space="PSUM"))
        _b = ctx.enter_context(tc.tile_pool(name="ps_b", bufs=PS_B, space="PSUM"))
        ps_l1 = [_l1] * CH
        ps_b = [_b] * CH

        def load(dr, shape, dt=F32R):
            t_ = singles.tile(shape, dt, tag=dr.name, name="s_" + dr.name)
            nc.sync.dma_start(t_[:], dr.ap())
            return t_

        s_wo1 = load(wo1, [DL, NU]); s_wo2b = load(wo2b, [NU + 1, DL])
        s_wz1y = load(wz1y, [DL, NU]); s_wz1x = load(wz1x, [DD, NU])
        s_wr1y = load(wr1y, [DL, NU]); s_wr1x = load(wr1x, [DD, NU])
        s_wh1y = load(wh1y, [DL, NU]); s_wh1x = load(wh1x, [DD, NU])
        s_wz2 = load(wz2, [NU, DL], BF16)
        s_wr2 = load(wr2, [NU, DL]); s_wh2 = load(wh2, [NU, DL])
        s_bo1 = load(bo1, [NU, 1], F32); s_bz1 = load(bz1, [NU, 1], F32)
        s_br1 = load(br1, [NU, 1], F32); s_bh1 = load(bh1, [NU, 1], F32)
        s_bzr2 = load(bzr2, [2 * DL, 1], F32); s_bh2 = load(bh2, [DL, 1], F32)
        s_dts = load(dts, [DL, NSTEP], F32)
        s_sgn = load(sgn, [2 * DL, 1], F32)

        # per-chunk persistent state tiles
        st = {}
        for c in range(CH):
            nch = CHS[c]
            cs = slice(COFF[c], COFF[c + 1])
            y = singles.tile([DL, nch], F32R, tag=f"y{c}", name=f"y{c}")
            nc.sync.dma_start(y[:], prior.ap()[:, cs])
            # rows 0:100 = tanh(ode layer1), row 100 = ones (folded bo2 row).
            # DMA the 32-aligned slab 96:128 with ones once; the per-step tanh
            # rewrites 96:100, row 100 stays 1.
            tode = singles.tile([128, nch], F32R, tag=f"tode{c}", name=f"tode{c}")
            nc.sync.dma_start(tode[96:128, 0:nch], ones.ap()[:, 0:nch])
            st[c] = dict(
                y=y, tode=tode,
                yode=singles.tile([DL, nch], F32R, tag=f"yode{c}", name=f"yode{c}"),
                rgy=singles.tile([DL, nch], F32R, tag=f"rgy{c}", name=f"rgy{c}"),
                tz=singles.tile([NU, nch], mybir.dt.bfloat16, tag=f"tz{c}", name=f"tz{c}"),
                tr=singles.tile([NU, nch], F32R, tag=f"tr{c}", name=f"tr{c}"),
                th=singles.tile([NU, nch], F32R, tag=f"th{c}", name=f"th{c}"),
                zr=singles.tile([2 * DL, nch], F32, tag=f"zr{c}", name=f"zr{c}"),
                zg0=singles.tile([DL, nch], F32, tag=f"zg0{c}", name=f"zg0{c}"),
                h=singles.tile([DL, nch], F32, tag=f"h{c}", name=f"h{c}"),
                d=singles.tile([DL, nch], F32, tag=f"d{c}", name=f"d{c}"),
                m=singles.tile([DL, nch], F32, tag=f"m{c}", name=f"m{c}"),
            )

        # Stage-interleaved emission: the chunk loop is INNERMOST per stage so
        # each in-order engine stream alternates chunks; while chunk A's next
        # op waits on its recurrence dependency, chunk B's same-stage op right
        # behind it in the stream is already satisfied (avoids head-of-line
        # blocking in the sequencers).
        chunks = list(range(CH))
        for t in range(NSTEP):
            xt = {}
            p1 = {}; p2 = {}; pz = {}; pr = {}; pzr = {}; ph = {}; ph2 = {}
            for c in chunks:
                cs = slice(COFF[c], COFF[c + 1])
                xt[c] = xpool.tile([DD, CHS[c]], F32R, tag="x", name="xt")
                nc.sync.dma_start(xt[c][:], xs.ap()[t, :, cs])
            for c in chunks:
                s = st[c]
                # --- ODE half-step ---
                p1[c] = ps_l1[c].tile([NU, CHS[c]], F32, tag="l1", name="p1")
                mmul(p1[c][:], s_wo1[:], s["y"][:])
            for c in chunks:
                s = st[c]
                nc.scalar.activation(s["tode"][0:NU, :], p1[c][:], TANH,
                                     bias=s_bo1[:, 0:1])
            for c in chunks:
                s = st[c]
                p2[c] = ps_b[c].tile([2 * DL, CHS[c]], F32, tag="b", name="p2")
                mmul(p2[c][0:DL, :], s_wo2b[:], s["tode"][0:NU + 1, :])
            for c in chunks:
                s = st[c]
                # y_ode = p2*dt + y   (bo2 folded into wo2b via ones row)
                nc.vector.scalar_tensor_tensor(
                    s["yode"][:], p2[c][0:DL, :], s_dts[:, t:t + 1],
                    s["y"][:].bitcast(F32), op0=MULT, op1=ADD)
            # --- z / r gate layer 1 (x-part emitted first: no recurrence dep)
            for c in chunks:
                pz[c] = ps_l1[c].tile([NU, CHS[c]], F32, tag="l1", name="pz")
                mmul(pz[c][:], s_wz1x[:], xt[c][:], start=True, stop=False)
                pr[c] = ps_l1[c].tile([NU, CHS[c]], F32, tag="l1", name="pr")
                mmul(pr[c][:], s_wr1x[:], xt[c][:], start=True, stop=False)
            for c in chunks:
                s = st[c]
                mmul(pr[c][:], s_wr1y[:], s["yode"][:], start=False, stop=True)
                mmul(pz[c][:], s_wz1y[:], s["yode"][:], start=False, stop=True)
            for c in chunks:
                s = st[c]
                nc.scalar.activation(s["tr"][:], pr[c][:], TANH, bias=s_br1[:, 0:1])
            for c in chunks:
                s = st[c]
                nc.scalar.activation(s["tz"][:], pz[c][:], TANH, bias=s_bz1[:, 0:1])
            # r gate in rows 0:64 (keeps the critical r->h path at base
            # partition 0 for the rgy mul); z in rows 64:128, copied to a
            # base-0 tile off the critical path (DVE/gpsimd tensor-tensor
            # ops need both SBUF operands at the same base partition).
            for c in chunks:
                s = st[c]
                pzr[c] = ps_b[c].tile([2 * DL, CHS[c]], F32, tag="b", name="pzr")
                mmul(pzr[c][0:DL, :], s_wr2[:], s["tr"][:])
                mmul(pzr[c][DL:2 * DL, :], s_wz2[:], s["tz"][:])
            for c in chunks:
                s = st[c]
                nc.scalar.activation(s["zr"][:], pzr[c][:], SIG,
                                     bias=s_bzr2[:, 0:1])
            # --- h gate ---
            for c in chunks:
                s = st[c]
                nc.vector.tensor_mul(s["rgy"][:], s["zr"][0:DL, :],
                                     s["yode"][:].bitcast(F32))
                nc.gpsimd.tensor_copy(s["zg0"][:], s["zr"][DL:2 * DL, :])
            for c in chunks:
                s = st[c]
                ph[c] = ps_l1[c].tile([NU, CHS[c]], F32, tag="l1", name="ph")
                mmul(ph[c][:], s_wh1x[:], xt[c][:], start=True, stop=False)
                mmul(ph[c][:], s_wh1y[:], s["rgy"][:], start=False, stop=True)
            for c in chunks:
                s = st[c]
                nc.scalar.activation(s["th"][:], ph[c][:], TANH,
                                     bias=s_bh1[:, 0:1])
            for c in chunks:
                s = st[c]
                ph2[c] = ps_b[c].tile([2 * DL, CHS[c]], F32, tag="b", name="ph2")
                mmul(ph2[c][0:DL, :], s_wh2[:], s["th"][:])
            for c in chunks:
                s = st[c]
                nc.scalar.activation(s["h"][:], ph2[c][0:DL, :], TANH,
                                     bias=s_bh2[:, 0:1])
            # --- GRU combine: y = h + z*(y_ode - h) ---
            for c in chunks:
                s = st[c]
                nc.gpsimd.tensor_sub(s["d"][:], s["yode"][:].bitcast(F32),
                                     s["h"][:])
            for c in chunks:
                s = st[c]
                nc.vector.tensor_mul(s["m"][:], s["zg0"][:], s["d"][:])
            for c in chunks:
                s = st[c]
                nc.vector.tensor_add(s["y"][:], s["h"][:], s["m"][:])

        for c in range(CH):
            cs = slice(COFF[c], COFF[c + 1])
            nc.sync.dma_start(yout.ap()[:, cs], st[c]["y"][:])

    nc.compile()
    return nc


_NC_CACHE = None


def _get_nc():
    global _NC_CACHE
    if _NC_CACHE is None:
        _NC_CACHE = _build()
    return _NC_CACHE


def _prep_core_inputs(data, time_steps, prior, weights):
    """Host-side glue: shard + transpose into the kernel's layouts."""
    dts = np.concatenate([time_steps[1:2] - time_steps[0:1],
                          time_steps[:-2] - time_steps[1:-1]]).astype(np.float32)
    dts_b = np.ascontiguousarray(
        np.broadcast_to(dts[None, :], (DL, NSTEP))).astype(np.float32)
    (Wo1, bo1, Wo2, bo2, Wz1, bz1, Wz2, bz2,
     Wr1, br1, Wr2, br2, Wh1, bh1, Wh2, bh2) = weights
    wo2b = np.concatenate([Wo2, bo2[None, :]], axis=0)
    shared = {
        "wo1": Wo1, "wo2b": wo2b,
        "wz1y": Wz1[:DL], "wz1x": Wz1[DL:], "wz2": Wz2,
        "wr1y": Wr1[:DL], "wr1x": Wr1[DL:], "wr2": Wr2,
        "wh1y": Wh1[:DL], "wh1x": Wh1[DL:], "wh2": Wh2,
        "bo1": bo1[:, None], "bz1": bz1[:, None],
        "br1": br1[:, None], "bh1": bh1[:, None],
        "bzr2": np.concatenate([br2, bz2])[:, None], "bh2": bh2[:, None],
        "sgn": np.concatenate([np.ones(DL, np.float32),
                               -np.ones(DL, np.float32)])[:, None],
        "dts": dts_b,
        "ones": np.ones((32, NCH), np.float32),
    }
    import ml_dtypes
    shared = {k: np.ascontiguousarray(v, dtype=np.float32)
              for k, v in shared.items()}
    shared["wz2"] = shared["wz2"].astype(ml_dtypes.bfloat16)
    in_maps = []
    for i in range(NCORES):
        ts_ = slice(i * B, (i + 1) * B)
        xs = np.ascontiguousarray(
            data[ts_, 1:, :].transpose(1, 2, 0)).astype(np.float32)
        pr = np.ascontiguousarray(prior[ts_].T).astype(np.float32)
        in_maps.append({"xs": xs, "prior": pr, **shared})
    return in_maps


def kernel(data, time_steps, prior,
           Wo1, bo1, Wo2, bo2,
           Wz1, bz1, Wz2, bz2,
           Wr1, br1, Wr2, br2,
           Wh1, bh1, Wh2, bh2):
    data = np.asarray(data, dtype=np.float32)
    time_steps = np.asarray(time_steps, dtype=np.float32)
    prior = np.asarray(prior, dtype=np.float32)
    weights = [np.asarray(w, dtype=np.float32) for w in
               (Wo1, bo1, Wo2, bo2, Wz1, bz1, Wz2, bz2,
                Wr1, br1, Wr2, br2, Wh1, bh1, Wh2, bh2)]
    nc = _get_nc()
    in_maps = _prep_core_inputs(data, time_steps, prior, weights)
    res = run_bass_kernel_spmd(nc, in_maps, core_ids=list(range(NCORES)))
    out = np.empty((N_TRAJ, DL), dtype=np.float32)
    for i in range(NCORES):
        out[i * B:(i + 1) * B] = res.results[i]["yout"].T
    return out

